# revision 1
# baseline (speedup 1.0000x reference)
"""GAT layer (gnn_message_passing) on 8 Trainium2 NeuronCores — v2.

Strategy (edge-parallel, dst-sharded, no collectives):
  - Host assigns each node to a "slot" (392 windows x 128 lids = 50176
    slots, 8 cores x 6272) balancing in-edges per window (greedy LPT),
    so per-window chunk counts are uniform across cores and padding is
    minimal. x is permuted into slot order on the host; the output is
    un-permuted at the end.
  - Phase 1 (per core, replicated): one matmul per 128 slots against a
    host-built rhs [W^T | W^T a_src | W^T a_dst] (bf16) produces h, s, d
    per slot; rows [h(128)|s|d|pad] (256 bf16 = 512B) go to a DRAM table.
  - Phase 2 (per core), windows in groups of 4:
      * SWDGE dma_gather of table rows for every edge's src slot with
        prepare_only=True + trigger_dma round-robined over 4 queues, so
        the Pool engine only generates descriptors and the transfers
        ride the 16 DMA engines asynchronously. s[src] arrives packed in
        the gathered row.
      * one per-element indirect DMA fetches d[dst] for every edge slot
        (offset = dstslot*256 + 129 into the table),
      * per group: one DVE add (s+d) and one tiny scalar-engine sigmoid
        produce sigma per edge,
      * per 128-edge chunk: ONE fused DVE tensor_scalar
        ST[e,l] = (iota_l == lid_e) * sigma_e, then
        matmul(psum += ST^T @ h_src) accumulates the window's weighted
        segment sum in PSUM; PSUM -> DMA to the output rows.
"""

import heapq
import os
from contextlib import ExitStack

import numpy as np
import ml_dtypes

import concourse.bass as bass
import concourse.bacc as bacc
import concourse.mybir as mybir
import concourse.tile as tile
from concourse.bass_utils import run_bass_kernel_spmd

N_NODES = 50000
N_EDGES = 800000
D = 128
CORES = 8
WIN = 128
NW = 49                       # windows per core
NWT = CORES * NW              # 392 windows total
NSLOT = NWT * WIN             # 50176 slots
NPC = NW * WIN                # 6272 slots per core
HALF = 32768                  # int16 gather addressing limit (slots)
NLOW = HALF // WIN            # 256 low windows
ROW = 256                     # table row elems (bf16): [h(128)|s|d|pad]
S_OFF = 128                   # s position within row
D_OFF = 129                   # d position within row
NODE_B = 512                  # phase-1 block (4 matmul subtiles)
WG = 4                        # windows per gather group
PREP_CH = int(os.environ.get("GAT2_PREPCH", "8"))
NQ = int(os.environ.get("GAT2_NQ", "4"))   # SWDGE queues
SCRATCH = int(os.environ.get("GAT2_SCRATCH", "16384"))
ABLATE = set(x for x in os.environ.get("GAT2_ABLATE", "").split(",") if x)

F32 = mybir.dt.float32
BF16 = mybir.dt.bfloat16
I32 = mybir.dt.int32
I16 = mybir.dt.int16
BF = ml_dtypes.bfloat16


class Plan:
    """Static chunk structure shared by host arrays and device program."""

    def __init__(self, ma, mb):
        self.ma = ma  # [NW] A-chunks per window (uniform across cores)
        self.mb = mb
        self.groups = []
        tot_ch = 0
        for g0 in range(0, NW, WG):
            wins = list(range(g0, min(g0 + WG, NW)))
            na = sum(int(ma[w]) for w in wins)
            nb = sum(int(mb[w]) for w in wins)
            a_rng, b_rng = {}, {}
            c = 0
            for w in wins:
                a_rng[w] = (c, c + int(ma[w]))
                c += int(ma[w])
            for w in wins:
                b_rng[w] = (c, c + int(mb[w]))
                c += int(mb[w])
            self.groups.append(dict(
                wins=wins, na=na, nb=nb, nch=na + nb,
                a_rng=a_rng, b_rng=b_rng, ch_base=tot_ch,
            ))
            tot_ch += na + nb
        self.tot_ch = tot_ch


def _assign_slots(src, dst):
    """node -> slot permutation balancing in-edges per window.

    Greedy LPT by in-degree into 392 windows (capacity 128), then
    windows are dealt to (core, local-k) positions sorted by A-edge
    count so the max-over-cores chunk count stays tight.
    """
    deg = np.bincount(dst, minlength=N_NODES).astype(np.int64)
    order = np.argsort(-deg, kind="stable")
    heap = [(0, w) for w in range(NWT)]
    heapq.heapify(heap)
    load = np.zeros(NWT, np.int64)
    fill = np.zeros(NWT, np.int32)
    win_of = np.empty(N_NODES, np.int32)
    for v in order:
        while True:
            _, w = heapq.heappop(heap)
            if fill[w] < WIN:
                break
        win_of[v] = w
        fill[w] += 1
        load[w] += deg[v]
        if fill[w] < WIN:
            heapq.heappush(heap, (load[w], w))

    # window ids 0..NLOW-1 are the low half (their slots are < 32768)
    src_low = win_of[src] < NLOW
    nA = np.bincount(win_of[dst][src_low], minlength=NWT)

    # position p = core*NW + k; p < NLOW is low. Deal windows sorted by
    # nA desc into positions ordered by (k, core) within each half, so
    # the 8 windows sharing a local k have similar A-counts.
    pos_order_low = [c * NW + k for k in range(NW) for c in range(CORES)
                     if c * NW + k < NLOW]
    pos_order_high = [c * NW + k for k in range(NW) for c in range(CORES)
                      if c * NW + k >= NLOW]
    ids_low = sorted(range(NLOW), key=lambda w: -nA[w])
    ids_high = sorted(range(NLOW, NWT), key=lambda w: -nA[w])
    pi = np.empty(NWT, np.int64)  # window id -> position
    for w, p in zip(ids_low, pos_order_low):
        pi[w] = p
    for w, p in zip(ids_high, pos_order_high):
        pi[w] = p

    # lid: rank of node within its window (by node id)
    order2 = np.lexsort((np.arange(N_NODES), win_of))
    lid = np.empty(N_NODES, np.int64)
    wsorted = win_of[order2]
    starts = np.searchsorted(wsorted, np.arange(NWT))
    ranks = np.arange(N_NODES) - starts[wsorted]
    lid[order2] = ranks

    slot = (pi[win_of] * WIN + lid).astype(np.int64)
    assert slot.min() >= 0 and slot.max() < NSLOT
    assert np.unique(slot).size == N_NODES
    return slot


def _preprocess(src, dst):
    src = np.ascontiguousarray(np.asarray(src, dtype=np.int64))
    dst = np.ascontiguousarray(np.asarray(dst, dtype=np.int64))
    slot = _assign_slots(src, dst)
    es = slot[src]
    ed = slot[dst]
    part = (es >= HALF).astype(np.int64)
    core = ed // NPC
    k = (ed % NPC) // WIN
    lid = (ed % WIN).astype(np.float64)

    bucket = (core * NW + k) * 2 + part
    eorder = np.argsort(bucket, kind="stable")
    counts = np.bincount(bucket, minlength=CORES * NW * 2).reshape(CORES, NW, 2)
    ma = -(-counts[:, :, 0].max(axis=0) // WIN)
    mb = -(-counts[:, :, 1].max(axis=0) // WIN)
    assert ((ma + mb) >= 1).all()
    plan = Plan(ma, mb)

    T = plan.tot_ch
    idx16 = np.zeros((CORES, 128, T * 8), np.int16)
    lids = np.full((CORES, 128, T), -1.0, np.float32)
    wbase = np.zeros((CORES, 128, NW), np.int32)
    for c in range(CORES):
        wbase[c, :, :] = (c * NPC + np.arange(NW) * WIN)[None, :]

    bstart = np.concatenate([[0], np.cumsum(counts.reshape(-1))])
    for c in range(CORES):
        for g in plan.groups:
            for p, rngs, shift in ((0, g["a_rng"], 0), (1, g["b_rng"], HALF)):
                for w in g["wins"]:
                    c0, c1 = rngs[w]
                    m = c1 - c0
                    if m == 0:
                        continue
                    b = (c * NW + w) * 2 + p
                    cnt = int(counts[c, w, p])
                    sel = eorder[bstart[b]: bstart[b] + cnt]
                    cap = m * WIN
                    fi = np.zeros(cap, np.int16)
                    fl = np.full(cap, -1.0, np.float32)
                    fi[:cnt] = (es[sel] - shift).astype(np.int16)
                    fl[:cnt] = lid[sel]
                    ch0 = g["ch_base"] + c0
                    lids[c, :, ch0: ch0 + m] = fl.reshape(m, WIN).T
                    wr = fi.reshape(cap // 16, 16).T  # [16, m*8]
                    idx16[c, :, ch0 * 8: (ch0 + m) * 8] = np.tile(wr, (8, 1))
    return slot, plan, idx16, lids, wbase


def _build_program(plan):
    nc = bacc.Bacc("TRN2", num_swdge_queues=NQ,
                   dynamic_dma_scratch_size=SCRATCH)

    xTp = nc.declare_dram_parameter("xTp", [D, NSLOT], BF16, isOutput=False)
    wsd = nc.declare_dram_parameter("wsd", [D, D + 2], BF16, isOutput=False)
    idx16 = nc.declare_dram_parameter("idx16", [128, plan.tot_ch * 8], I16,
                                      isOutput=False)
    lids = nc.declare_dram_parameter("lids", [128, plan.tot_ch], F32,
                                     isOutput=False)
    wbase = nc.declare_dram_parameter("wbase", [128, NW], I32, isOutput=False)
    F = nc.declare_dram_parameter("F", [NPC, D], F32, isOutput=True)
    table = nc.dram_tensor("table", [NSLOT, ROW], BF16)
    d_arr = nc.dram_tensor("d_arr", [NSLOT, 1], F32)

    with tile.TileContext(nc) as tc, ExitStack() as stack:
        const = stack.enter_context(tc.tile_pool(name="const", bufs=1))
        xt_pool = stack.enter_context(tc.tile_pool(name="xt", bufs=3))
        stage_pool = stack.enter_context(tc.tile_pool(name="stage", bufs=3))
        hg_pool = stack.enter_context(tc.tile_pool(name="hg", bufs=2))
        sg_pool = stack.enter_context(tc.tile_pool(name="sg", bufs=3))
        st_pool = stack.enter_context(tc.tile_pool(name="st", bufs=6))
        out_pool = stack.enter_context(tc.tile_pool(name="out", bufs=3))
        ps1_pool = stack.enter_context(tc.tile_pool(name="ps1", bufs=3, space="PSUM"))
        psd_pool = stack.enter_context(tc.tile_pool(name="psd", bufs=1, space="PSUM"))
        psw_pool = stack.enter_context(tc.tile_pool(name="psw", bufs=4, space="PSUM"))

        # ---- setup ----
        from concourse.masks import make_identity
        ident = const.tile([128, 128], F32)
        make_identity(nc, ident[:])
        iota = const.tile([128, 128], F32)
        nc.gpsimd.iota(iota[:], pattern=[[1, 128]], base=0,
                       channel_multiplier=0,
                       allow_small_or_imprecise_dtypes=True)
        wsd_sb = const.tile([128, D + 2], BF16)
        nc.sync.dma_start(out=wsd_sb[:], in_=wsd[:, :])
        it = const.tile([128, plan.tot_ch * 8], I16)
        nc.sync.dma_start(out=it[:], in_=idx16[:, :])
        lt = const.tile([128, plan.tot_ch], F32)
        nc.sync.dma_start(out=lt[:], in_=lids[:, :])
        wb = const.tile([128, NW], I32)
        nc.sync.dma_start(out=wb[:], in_=wbase[:, :])

        # ---- phase 1: table rows [h | s | d | pad] ----
        n_blocks = NSLOT // NODE_B
        nsub = NODE_B // 128
        for t in range(n_blocks):
            r0 = t * NODE_B
            xt = xt_pool.tile([128, NODE_B], BF16)
            nc.scalar.dma_start(out=xt[:], in_=xTp[:, r0:r0 + NODE_B])
            stage = stage_pool.tile([128, nsub * (D + 2)], BF16)
            dstage = stage_pool.tile([128, nsub], F32, tag="dstage")
            for s in range(nsub):
                ps = ps1_pool.tile([128, D + 2], F32)
                nc.tensor.matmul(out=ps[:], lhsT=xt[:, s * 128:(s + 1) * 128],
                                 rhs=wsd_sb[:], start=True, stop=True)
                dst_ap = stage[:, s * (D + 2): (s + 1) * (D + 2)]
                if s % 2 == 0:
                    nc.vector.tensor_copy(out=dst_ap, in_=ps[:])
                    nc.vector.tensor_copy(out=dstage[:, s:s + 1],
                                          in_=ps[:, D + 1:D + 2])
                else:
                    nc.scalar.copy(out=dst_ap, in_=ps[:])
                    nc.scalar.copy(out=dstage[:, s:s + 1],
                                   in_=ps[:, D + 1:D + 2])
            dT_ps = psd_pool.tile([nsub, 128], F32, tag="dT")
            nc.tensor.transpose(out=dT_ps[:], in_=dstage[:], identity=ident[:])
            dT_sb = stage_pool.tile([nsub, 128], F32, tag="dTs")
            nc.vector.tensor_copy(out=dT_sb[:], in_=dT_ps[:])
            nc.scalar.dma_start(
                out=d_arr[r0:r0 + NODE_B, :].rearrange(
                    "(j e) one -> j (e one)", e=128),
                in_=dT_sb[:])
            nc.sync.dma_start(
                out=table[r0:r0 + NODE_B, 0:D + 2].rearrange(
                    "(j p) e -> p j e", p=128),
                in_=stage[:].rearrange("p (j e) -> p j e", e=D + 2))

        # ---- phase 2: gather + weighted segment sum ----
        tblA = table[0:HALF, :]
        tblB = table[HALF:NSLOT, :]
        qrr = [0]

        def gather(out_ap, idxs_ap, n_idx, base_ap):
            q = qrr[0]
            qrr[0] = (q + 1) % NQ
            nc.gpsimd.dma_gather(
                out_ap=out_ap, in_ap=base_ap, idxs_ap=idxs_ap,
                num_idxs=n_idx, num_idxs_reg=n_idx, elem_size=ROW,
                queue_num=q)

        if "ph2" in ABLATE:
            zt = const.tile([128, D], F32)
            nc.vector.memset(zt[:], 0.0)
            for w in range(NW):
                nc.sync.dma_start(out=F[w * WIN:(w + 1) * WIN, :], in_=zt[:])
        groups2 = [] if "ph2" in ABLATE else plan.groups
        for g in groups2:
            nch = g["nch"]
            cb = g["ch_base"]
            hg = hg_pool.tile([128, nch * ROW], BF16)
            hg3 = hg[:].rearrange("p (c e) -> p c e", e=ROW)
            for p, n_part, base_ap, ch0 in (
                (0, g["na"], tblA, 0), (1, g["nb"], tblB, g["na"]),
            ):
                for s0 in range(0, n_part, PREP_CH):
                    sn = min(PREP_CH, n_part - s0)
                    c0 = ch0 + s0
                    gather(hg3[:, c0:c0 + sn, :],
                           it[:, (cb + c0) * 8:(cb + c0 + sn) * 8],
                           sn * 128, base_ap)

            gw = len(g["wins"])
            w0 = g["wins"][0]
            db = sg_pool.tile([128, gw * WIN], F32, tag="db")
            nc.gpsimd.indirect_dma_start(
                out=db[:], out_offset=None, in_=d_arr[:, :],
                in_offset=bass.IndirectOffsetOnAxis(
                    ap=wb[:, w0:w0 + 1], axis=0))

            for wloc, w in enumerate(g["wins"]):
                chunks = (list(range(*g["a_rng"][w])) +
                          list(range(*g["b_rng"][w])))
                psw = psw_pool.tile([128, D], F32)
                for j, c in enumerate(chunks):
                    sigt = st_pool.tile([128, 128], BF16, tag="sig")
                    nc.scalar.activation(
                        out=sigt[:], in_=db[:, wloc * WIN:(wloc + 1) * WIN],
                        func=mybir.ActivationFunctionType.Sigmoid,
                        bias=hg3[:, c, S_OFF:S_OFF + 1], scale=1.0)
                    st = st_pool.tile([128, 128], BF16)
                    nc.vector.scalar_tensor_tensor(
                        out=st[:], in0=iota[:],
                        scalar=lt[:, cb + c:cb + c + 1],
                        in1=sigt[:],
                        op0=mybir.AluOpType.is_equal,
                        op1=mybir.AluOpType.mult)
                    nc.tensor.matmul(out=psw[:], lhsT=st[:],
                                     rhs=hg3[:, c, 0:D],
                                     start=(j == 0), stop=(j == len(chunks) - 1))
                out_t = out_pool.tile([128, D], F32)
                if w % 2 == 0:
                    nc.scalar.copy(out=out_t[:], in_=psw[:])
                else:
                    nc.vector.tensor_copy(out=out_t[:], in_=psw[:])
                nc.sync.dma_start(out=F[w * WIN:(w + 1) * WIN, :], in_=out_t[:])

    nc.finalize()
    return nc


def _host_arrays(x, W, a, slot):
    x = np.asarray(x, dtype=np.float32)
    W = np.asarray(W, dtype=np.float32)
    a = np.asarray(a, dtype=np.float32)
    a_src = a[0, :D]
    a_dst = a[0, D:]
    # xTp[f, slot] = x[node, f]  (zeros for empty slots)
    xp = np.zeros((NSLOT, D), np.float32)
    xp[slot] = x
    xTp = np.ascontiguousarray(xp.T.astype(BF))
    wsd = np.concatenate(
        [W.T, (W.T @ a_src)[:, None], (W.T @ a_dst)[:, None]], axis=1)
    wsd = np.ascontiguousarray(wsd.astype(BF))
    return xTp, wsd


def _run(x, W, a, src, dst, trace=False, trace_cores=None):
    slot, plan, idx16, lids, wbase = _preprocess(src, dst)
    xTp, wsd = _host_arrays(x, W, a, slot)
    nc = _build_program(plan)
    in_maps = [
        {"xTp": xTp, "wsd": wsd,
         "idx16": np.ascontiguousarray(idx16[c]),
         "lids": np.ascontiguousarray(lids[c]),
         "wbase": np.ascontiguousarray(wbase[c])}
        for c in range(CORES)
    ]
    res = run_bass_kernel_spmd(nc, in_maps, list(range(CORES)),
                               trace=trace, trace_cores=trace_cores)
    F_all = np.concatenate([np.asarray(res.results[c]["F"]) for c in range(CORES)],
                           axis=0)
    out = np.ascontiguousarray(F_all[slot].astype(np.float32))
    return out, res


def kernel(x, W, a, src, dst):
    out, _ = _run(x, W, a, src, dst)
    return out



# revision 13
# speedup vs baseline: 1.0484x; 1.0484x over previous
"""GAT layer (gnn_message_passing) on 8 Trainium2 NeuronCores — v3.

Strategy (edge-parallel, dst-sharded, no collectives):
  - Host assigns each node to a "slot" (392 windows x 128 lids, 8 cores x
    6272 slots) balancing in-edges per window (greedy LPT by in-degree);
    within a window, nodes are dealt to the two 64-lid sub-windows by
    degree for balance. x is permuted into slot order on the host; the
    output is un-permuted at the end.
  - Table rows are 256B: [h_perm(127 bf16) | s bf16], where h_perm drops
    the feature f* = argmax|a_src| (best conditioning). s = h@a_src is a
    linear combination of h, so the dropped output column is recovered on
    the host from the per-node sigma-weighted s sums that the main matmul
    produces for free in column 127. 256B rows halve the SWDGE
    per-descriptor cost vs 512B (HW-measured 3.17 vs 6.71 ns/desc).
  - Phase 1 (per core, replicated): per 128 slots one matmul against a
    host-built rhs [W^T_perm | W^T a_src | W^T a_dst] (bf16) produces
    [h'|s|d]; [h'|s] rows go to the DRAM table as full contiguous 256B
    rows (64KB contiguous runs per 128 slots), spread over 3 DMA queues;
    d columns accumulate in SBUF and are transposed once at the end into
    d_arr (bf16).
  - Phase 2 (per core), windows in groups of 4:
      * SWDGE dma_gather of 256B table rows for every edge's src slot,
        4 queues round-robin; s[src] arrives in row column 127.
      * one indirect DMA per group broadcasts d of the group's dst
        windows into db [128, gw*128] bf16.
      * chunks are bucketed by (window, 64-lid sub-window, src-half):
        per chunk one DVE tensor_scalar tmp = d_sub + s (bf16, fast
        mode), per bucket ONE scalar-engine sigmoid over the batched
        tmp, per chunk one DVE scalar_tensor_tensor building the masked
        scatter matrix ST[e,l] = (iota64==lid)*sigma (all bf16), and one
        matmul psum[64s:64s+64] += ST^T @ h_rows.
      * per window: PSUM -> SBUF copy -> DMA to the output rows.
  - Host: un-permute slots, un-permute features, reconstruct column f*:
    F[:,f*] = (F'[:,127] - sum_j a_perm[j] F'[:,j]) / a_src[f*].
"""

import heapq
import os
from contextlib import ExitStack

import numpy as np
import ml_dtypes

import concourse.bass as bass
import concourse.bacc as bacc
import concourse.mybir as mybir
import concourse.tile as tile
from concourse.bass_utils import run_bass_kernel_spmd

N_NODES = 50000
N_EDGES = 800000
D = 128
CORES = 8
WIN = 128
SUB = 64                      # lids per sub-window
NSUBW = WIN // SUB            # sub-windows per window (2)
NW = 49                       # windows per core
NWT = CORES * NW              # 392 windows total
NSLOT = NWT * WIN             # 50176 slots
NPC = NW * WIN                # 6272 slots per core
HALF = 32768                  # int16 gather addressing limit (slots)
ROW = 128                     # table row elems (bf16): [h_perm(127)|s]
S_OFF = 127                   # s position within row
NODE_B = 512                  # phase-1 block (4 matmul subtiles)
WG = 4                        # windows per gather group
PREP_CH = int(os.environ.get("GAT3_PREPCH", "8"))
NQ = int(os.environ.get("GAT3_NQ", "4"))   # SWDGE queues
SCRATCH = int(os.environ.get("GAT3_SCRATCH", "16384"))
ABLATE = set(x for x in os.environ.get("GAT3_ABLATE", "").split(",") if x)

F32 = mybir.dt.float32
BF16 = mybir.dt.bfloat16
I32 = mybir.dt.int32
I16 = mybir.dt.int16
BF = ml_dtypes.bfloat16


class Plan:
    """Static chunk structure shared by host arrays and device program.

    m[w][s][p] chunks per (window, sub-window, src-half) bucket, uniform
    across cores. Chunk layout within a group: all A-chunks (w-major,
    sub-minor), then all B-chunks.
    """

    def __init__(self, m):
        self.m = m  # [NW, NSUBW, 2]
        self.groups = []
        tot_ch = 0
        for g0 in range(0, NW, WG):
            wins = list(range(g0, min(g0 + WG, NW)))
            rng = {}
            c = 0
            for p in (0, 1):
                for w in wins:
                    for s in range(NSUBW):
                        mm = int(m[w, s, p])
                        rng[(w, s, p)] = (c, c + mm)
                        c += mm
            na = sum(int(m[w, s, 0]) for w in wins for s in range(NSUBW))
            nb = sum(int(m[w, s, 1]) for w in wins for s in range(NSUBW))
            self.groups.append(dict(
                wins=wins, na=na, nb=nb, nch=na + nb,
                rng=rng, ch_base=tot_ch,
            ))
            tot_ch += na + nb
        self.tot_ch = tot_ch


def _assign_slots(src, dst):
    """node -> slot permutation balancing in-edges per window.

    Greedy LPT by in-degree into 392 windows (capacity 128; windows 0 and
    NWT-1 capped at 127 so one slot in each int16-half stays empty as an
    all-zero dummy gather target), then windows are dealt to (core,
    local-k) positions sorted by A-edge count. Within a window, nodes are
    dealt to the two 64-lid sub-windows by degree order for balance.
    """
    deg = np.bincount(dst, minlength=N_NODES).astype(np.int64)
    order = np.argsort(-deg, kind="stable")
    cap = np.full(NWT, WIN, np.int32)
    cap[0] = WIN - 1          # low-half dummy (window id 0 -> A position)
    cap[NWT - 1] = WIN - 1    # high-half dummy (id NWT-1 -> B position)
    heap = [(0, w) for w in range(NWT)]
    heapq.heapify(heap)
    load = np.zeros(NWT, np.int64)
    fill = np.zeros(NWT, np.int32)
    win_of = np.empty(N_NODES, np.int32)
    for v in order:
        while True:
            _, w = heapq.heappop(heap)
            if fill[w] < cap[w]:
                break
        win_of[v] = w
        fill[w] += 1
        load[w] += deg[v]
        if fill[w] < cap[w]:
            heapq.heappush(heap, (load[w], w))

    # window ids 0..NLOW-1 are the low half (their slots are < 32768)
    NLOW = HALF // WIN
    src_low = win_of[src] < NLOW
    nA = np.bincount(win_of[dst][src_low], minlength=NWT)

    pos_order_low = [c * NW + k for k in range(NW) for c in range(CORES)
                     if c * NW + k < NLOW]
    pos_order_high = [c * NW + k for k in range(NW) for c in range(CORES)
                      if c * NW + k >= NLOW]
    ids_low = sorted(range(NLOW), key=lambda w: -nA[w])
    ids_high = sorted(range(NLOW, NWT), key=lambda w: -nA[w])
    pi = np.empty(NWT, np.int64)  # window id -> position
    for w, p in zip(ids_low, pos_order_low):
        pi[w] = p
    for w, p in zip(ids_high, pos_order_high):
        pi[w] = p

    # lid: deal window's nodes (degree-desc) alternately to sub-windows
    lid = np.empty(N_NODES, np.int64)
    nodes_by_win = [[] for _ in range(NWT)]
    for v in order:
        nodes_by_win[win_of[v]].append(v)
    for w in range(NWT):
        for i, v in enumerate(nodes_by_win[w]):
            lid[v] = (i % NSUBW) * SUB + i // NSUBW

    slot = (pi[win_of] * WIN + lid).astype(np.int64)
    assert slot.min() >= 0 and slot.max() < NSLOT
    assert np.unique(slot).size == N_NODES
    used = np.zeros(NSLOT, bool)
    used[slot] = True
    emptyA = np.flatnonzero(~used[:HALF])
    emptyB = np.flatnonzero(~used[HALF:])
    assert emptyA.size > 0 and emptyB.size > 0
    dummyA = int(emptyA[0])
    dummyB = int(emptyB[0])  # relative to HALF
    return slot, dummyA, dummyB


def _preprocess(src, dst):
    src = np.ascontiguousarray(np.asarray(src, dtype=np.int64))
    dst = np.ascontiguousarray(np.asarray(dst, dtype=np.int64))
    slot, dummyA, dummyB = _assign_slots(src, dst)
    es = slot[src]
    ed = slot[dst]
    part = (es >= HALF).astype(np.int64)
    core = ed // NPC
    k = (ed % NPC) // WIN
    lid = ed % WIN
    sub = lid // SUB
    lid_rel = (lid % SUB).astype(np.float64)

    bucket = (((core * NW + k) * NSUBW + sub) * 2 + part)
    eorder = np.argsort(bucket, kind="stable")
    counts = np.bincount(bucket, minlength=CORES * NW * NSUBW * 2).reshape(
        CORES, NW, NSUBW, 2)
    mx = counts.max(axis=0)  # [NW, NSUBW, 2]
    m = -(-mx // WIN)
    m[:, :, 0] = np.maximum(m[:, :, 0], 1)  # ensure psum init per (w,s)
    plan = Plan(m)

    T = plan.tot_ch
    idx16 = np.zeros((CORES, 128, T * 8), np.int16)
    lids = np.full((CORES, 128, T), -1.0, np.float32)
    wbase = np.zeros((CORES, 128, NW), np.int32)
    for c in range(CORES):
        wbase[c, :, :] = (c * NPC + np.arange(NW) * WIN)[None, :]

    bstart = np.concatenate([[0], np.cumsum(counts.reshape(-1))])
    for c in range(CORES):
        for g in plan.groups:
            for w in g["wins"]:
                for s in range(NSUBW):
                    for p, shift, dmy in ((0, 0, dummyA), (1, HALF, dummyB)):
                        c0, c1 = g["rng"][(w, s, p)]
                        mm = c1 - c0
                        if mm == 0:
                            continue
                        b = ((c * NW + w) * NSUBW + s) * 2 + p
                        cnt = int(counts[c, w, s, p])
                        sel = eorder[bstart[b]: bstart[b] + cnt]
                        capn = mm * WIN
                        fi = np.full(capn, dmy, np.int16)
                        fl = np.full(capn, -1.0, np.float32)
                        fi[:cnt] = (es[sel] - shift).astype(np.int16)
                        fl[:cnt] = lid_rel[sel]
                        ch0 = g["ch_base"] + c0
                        lids[c, :, ch0: ch0 + mm] = fl.reshape(mm, WIN).T
                        wr = fi.reshape(capn // 16, 16).T  # [16, mm*8]
                        idx16[c, :, ch0 * 8: (ch0 + mm) * 8] = np.tile(wr, (8, 1))
    return slot, plan, idx16, lids, wbase


def _build_program(plan):
    nc = bacc.Bacc("TRN2", num_swdge_queues=NQ,
                   dynamic_dma_scratch_size=SCRATCH)

    xTp = nc.declare_dram_parameter("xTp", [D, NSLOT], BF16, isOutput=False)
    wsd = nc.declare_dram_parameter("wsd", [D, ROW + 1], BF16, isOutput=False)
    idx16 = nc.declare_dram_parameter("idx16", [128, plan.tot_ch * 8], I16,
                                      isOutput=False)
    lids = nc.declare_dram_parameter("lids", [128, plan.tot_ch], F32,
                                     isOutput=False)
    wbase = nc.declare_dram_parameter("wbase", [128, NW], I32, isOutput=False)
    F = nc.declare_dram_parameter("F", [NPC, D], F32, isOutput=True)
    table = nc.dram_tensor("table", [NSLOT, ROW], BF16)
    d_arr = nc.dram_tensor("d_arr", [NSLOT, 1], BF16)

    NBLK = NSLOT // NODE_B            # 98 phase-1 blocks
    nsub = NODE_B // 128              # 4
    NDC = NBLK * nsub                 # 392 d columns

    with tile.TileContext(nc) as tc, ExitStack() as stack:
        const = stack.enter_context(tc.tile_pool(name="const", bufs=1))
        xt_pool = stack.enter_context(tc.tile_pool(name="xt", bufs=3))
        stage_pool = stack.enter_context(tc.tile_pool(name="stage", bufs=3))
        hg_pool = stack.enter_context(tc.tile_pool(name="hg", bufs=2))
        sg_pool = stack.enter_context(tc.tile_pool(name="sg", bufs=3))
        tmp_pool = stack.enter_context(tc.tile_pool(name="tmp", bufs=4))
        st_pool = stack.enter_context(tc.tile_pool(name="st", bufs=8))
        out_pool = stack.enter_context(tc.tile_pool(name="out", bufs=3))
        ps1_pool = stack.enter_context(tc.tile_pool(name="ps1", bufs=3, space="PSUM"))
        psd_pool = stack.enter_context(tc.tile_pool(name="psd", bufs=1, space="PSUM"))
        psw_pool = stack.enter_context(tc.tile_pool(name="psw", bufs=4, space="PSUM"))

        # ---- setup ----
        from concourse.masks import make_identity
        ident = const.tile([128, 128], F32)
        make_identity(nc, ident[:])
        ident_bf = const.tile([128, 128], BF16)
        nc.vector.tensor_copy(out=ident_bf[:], in_=ident[:])
        iota = const.tile([128, SUB], BF16)
        nc.gpsimd.iota(iota[:], pattern=[[1, SUB]], base=0,
                       channel_multiplier=0,
                       allow_small_or_imprecise_dtypes=True)
        wsd_sb = const.tile([128, ROW + 1], BF16)
        nc.sync.dma_start(out=wsd_sb[:], in_=wsd[:, :])
        it = const.tile([128, plan.tot_ch * 8], I16)
        nc.sync.dma_start(out=it[:], in_=idx16[:, :])
        lt = const.tile([128, plan.tot_ch], F32)
        nc.sync.dma_start(out=lt[:], in_=lids[:, :])
        wb = const.tile([128, NW], I32)
        nc.sync.dma_start(out=wb[:], in_=wbase[:, :])
        stage_all = const.tile([128, NBLK * nsub * (ROW + 1)], BF16)
        stage3 = stage_all[:].rearrange("p (c e) -> p c e", e=ROW + 1)

        # ---- phase 1: table rows [h_perm | s], d columns ----
        if "ph1" not in ABLATE:
            for t in range(NBLK):
                r0 = t * NODE_B
                xt = xt_pool.tile([128, NODE_B], BF16)
                if t % 2 == 0:
                    nc.sync.dma_start(out=xt[:], in_=xTp[:, r0:r0 + NODE_B])
                else:
                    nc.gpsimd.dma_start(out=xt[:], in_=xTp[:, r0:r0 + NODE_B])
                for s in range(nsub):
                    ps = ps1_pool.tile([128, ROW + 1], F32)
                    nc.tensor.matmul(out=ps[:], lhsT=xt[:, s * 128:(s + 1) * 128],
                                     rhs=wsd_sb[:], start=True, stop=True)
                    col = t * nsub + s
                    dst_ap = stage_all[:, col * (ROW + 1): (col + 1) * (ROW + 1)]
                    if s % 2 == 0:
                        nc.vector.tensor_copy(out=dst_ap, in_=ps[:])
                    else:
                        nc.scalar.copy(out=dst_ap, in_=ps[:])
                wout = table[r0:r0 + NODE_B, :].rearrange("(j p) e -> p j e", p=128)
                win_ = stage3[:, t * nsub:(t + 1) * nsub, 0:ROW]
                if t % 3 == 0:
                    nc.gpsimd.dma_start(out=wout, in_=win_)
                elif t % 3 == 1:
                    nc.sync.dma_start(out=wout, in_=win_)
                else:
                    nc.scalar.dma_start(out=wout, in_=win_)

            # d_arr: transpose d columns (stage col 128) -> d[c*128+p]
            dcols = stage3[:, :, ROW:ROW + 1].rearrange("p c one -> p (c one)")
            for b0 in range(0, NDC, 128):
                bw = min(128, NDC - b0)
                dT_ps = psd_pool.tile([128, 128], BF16, tag="dT")
                nc.tensor.transpose(out=dT_ps[:bw, :], in_=dcols[:, b0:b0 + bw],
                                    identity=ident_bf[:])
                dT_sb = stage_pool.tile([128, 128], BF16, tag="dTs")
                nc.vector.tensor_copy(out=dT_sb[:bw, :], in_=dT_ps[:bw, :])
                nc.sync.dma_start(
                    out=d_arr[b0 * 128:(b0 + bw) * 128, :].rearrange(
                        "(c p) one -> c (p one)", p=128),
                    in_=dT_sb[:bw, :])

        # ---- phase 2: gather + weighted segment sum ----
        tblA = table[0:HALF, :]
        tblB = table[HALF:NSLOT, :]
        qrr = [0]

        def gather(out_ap, idxs_ap, n_idx, base_ap):
            q = qrr[0]
            qrr[0] = (q + 1) % NQ
            nc.gpsimd.dma_gather(
                out_ap=out_ap, in_ap=base_ap, idxs_ap=idxs_ap,
                num_idxs=n_idx, num_idxs_reg=n_idx, elem_size=ROW,
                queue_num=q)

        if "ph2" in ABLATE:
            zt = const.tile([128, D], F32)
            nc.vector.memset(zt[:], 0.0)
            for w in range(NW):
                nc.sync.dma_start(out=F[w * WIN:(w + 1) * WIN, :], in_=zt[:])
        groups2 = [] if "ph2" in ABLATE else plan.groups
        for g in groups2:
            nch = g["nch"]
            cb = g["ch_base"]
            hg = hg_pool.tile([128, nch * ROW], BF16)
            hg3 = hg[:].rearrange("p (c e) -> p c e", e=ROW)
            for p, n_part, base_ap, ch0 in (
                (0, g["na"], tblA, 0), (1, g["nb"], tblB, g["na"]),
            ):
                for s0 in range(0, n_part, PREP_CH):
                    sn = min(PREP_CH, n_part - s0)
                    c0 = ch0 + s0
                    gather(hg3[:, c0:c0 + sn, :],
                           it[:, (cb + c0) * 8:(cb + c0 + sn) * 8],
                           sn * 128, base_ap)

            gw = len(g["wins"])
            w0 = g["wins"][0]
            db = sg_pool.tile([128, gw * WIN], BF16, tag="db")
            nc.gpsimd.indirect_dma_start(
                out=db[:], out_offset=None, in_=d_arr[:, :],
                in_offset=bass.IndirectOffsetOnAxis(
                    ap=wb[:, w0:w0 + 1], axis=0))

            for wloc, w in enumerate(g["wins"]):
                psw = psw_pool.tile([128, D], F32)
                for s in range(NSUBW):
                    chunks = (list(range(*g["rng"][(w, s, 0)])) +
                              list(range(*g["rng"][(w, s, 1)])))
                    nck = len(chunks)
                    if nck == 0:
                        continue
                    tmpg = tmp_pool.tile([128, nck * SUB], BF16, tag="tmpg")
                    sigt = tmp_pool.tile([128, nck * SUB], BF16, tag="sigt")
                    db1 = db[:, wloc * WIN + s * SUB: wloc * WIN + (s + 1) * SUB
                             ].rearrange("p (one e) -> p one e", one=1)
                    j0 = 0
                    for pp in (0, 1):
                        pc0, pc1 = g["rng"][(w, s, pp)]
                        np_ = pc1 - pc0
                        if np_ == 0:
                            continue
                        nc.vector.tensor_tensor(
                            out=tmpg[:, j0 * SUB:(j0 + np_) * SUB].rearrange(
                                "p (c e) -> p c e", e=SUB),
                            in0=db1.to_broadcast([128, np_, SUB]),
                            in1=hg3[:, pc0:pc1, S_OFF:S_OFF + 1]
                                .to_broadcast([128, np_, SUB]),
                            op=mybir.AluOpType.add)
                        j0 += np_
                    nc.scalar.activation(
                        out=sigt[:], in_=tmpg[:],
                        func=mybir.ActivationFunctionType.Sigmoid)
                    for j, c in enumerate(chunks):
                        st = st_pool.tile([128, SUB], BF16)
                        nc.vector.scalar_tensor_tensor(
                            out=st[:], in0=iota[:],
                            scalar=lt[:, cb + c:cb + c + 1],
                            in1=sigt[:, j * SUB:(j + 1) * SUB],
                            op0=mybir.AluOpType.is_equal,
                            op1=mybir.AluOpType.mult)
                        nc.tensor.matmul(out=psw[:, s * SUB:(s + 1) * SUB],
                                         lhsT=hg3[:, c, 0:ROW], rhs=st[:],
                                         start=(j == 0), stop=(j == nck - 1))
                out_t = out_pool.tile([128, D], F32)
                if w % 2 == 0:
                    nc.scalar.copy(out=out_t[:], in_=psw[:])
                else:
                    nc.vector.tensor_copy(out=out_t[:], in_=psw[:])
                if w % 2 == 0:
                    nc.sync.dma_start(out=F[w * WIN:(w + 1) * WIN, :], in_=out_t[:])
                else:
                    nc.scalar.dma_start(out=F[w * WIN:(w + 1) * WIN, :], in_=out_t[:])

    nc.finalize()
    return nc


def _host_arrays(x, W, a, slot):
    x = np.asarray(x, dtype=np.float32)
    W = np.asarray(W, dtype=np.float32)
    a = np.asarray(a, dtype=np.float32)
    a_src = a[0, :D]
    a_dst = a[0, D:]
    fstar = int(np.argmax(np.abs(a_src)))
    perm = np.array([f for f in range(D) if f != fstar], np.int64)  # 127 cols
    # xTp[f, slot] = x[node, f]  (zeros for empty slots)
    xp = np.zeros((NSLOT, D), np.float32)
    xp[slot] = x
    xTp = np.ascontiguousarray(xp.T.astype(BF))
    WT = W.T  # [in, out]
    wsd = np.concatenate(
        [WT[:, perm], (WT @ a_src)[:, None], (WT @ a_dst)[:, None]], axis=1)
    wsd = np.ascontiguousarray(wsd.astype(BF))
    return xTp, wsd, fstar, perm, a_src


def _run(x, W, a, src, dst, trace=False, trace_cores=None):
    slot, plan, idx16, lids, wbase = _preprocess(src, dst)
    xTp, wsd, fstar, perm, a_src = _host_arrays(x, W, a, slot)
    nc = _build_program(plan)
    in_maps = [
        {"xTp": xTp, "wsd": wsd,
         "idx16": np.ascontiguousarray(idx16[c]),
         "lids": np.ascontiguousarray(lids[c]),
         "wbase": np.ascontiguousarray(wbase[c])}
        for c in range(CORES)
    ]
    res = run_bass_kernel_spmd(nc, in_maps, list(range(CORES)),
                               trace=trace, trace_cores=trace_cores)
    Fp = np.concatenate([np.asarray(res.results[c]["F"]) for c in range(CORES)],
                        axis=0)  # [NSLOT, 128], F^T per window: [w*128+f, lid]
    Fp = np.ascontiguousarray(
        Fp.reshape(NWT, WIN, D).transpose(0, 2, 1).reshape(NSLOT, D))
    Fn = Fp[slot]  # [N_NODES, 128]: cols 0..126 = h_perm, 127 = ss
    out = np.empty((N_NODES, D), np.float32)
    out[:, perm] = Fn[:, 0:D - 1]
    out[:, fstar] = (Fn[:, D - 1] - Fn[:, 0:D - 1] @ a_src[perm]) / a_src[fstar]
    return np.ascontiguousarray(out), res


def kernel(x, W, a, src, dst):
    out, _ = _run(x, W, a, src, dst)
    return out
